# revision 36
# baseline (speedup 1.0000x reference)
"""GQA attention kernel for Trainium2, tensor-parallel over 8 NeuronCores.

Sharding: 4 q-heads + 1 kv-group per core (H=32, G=8). Each core computes
its heads' attention and a partial out-projection; host sums the 8 partials
(the "all-reduce after out_proj").

Layout: everything transposed (head_dim on partitions, sequence on the free
dim) so Q/K projections, scores and ctx matmuls run with 512-wide moving
operands; no runtime transposes needed except V (16 small PE transposes).

All matmul operands are bf16 (tolerance is 2e-2; fp32 runs the PE in the
4x-slower fp32_mode=HIGH path); scores/ctx/out accumulate in fp32 PSUM.

The PE stream is kept dense (HAM stays un-throttled) by software-pipelining
cross-engine chains: the RMSNorm reduction/broadcast matmuls of chunk sc are
spliced into the middle of chunk sc+1's projection matmuls, and attention
out-projection tiles are interleaved into the exp-bound attention loop.

RMSNorm over head_dim (= partitions) is a ones-matmul partition reduction +
Sqrt + reciprocal_approx_fast + broadcast (matmul for q, gpsimd
partition_broadcast for k/attention — the custom ops need base partition 0);
RoPE's rotate_half is a 32-lane pairwise stream_shuffle (head-dim components
permuted host-side into (i, i+32) pairs; wo is NOT permuted since V/ctx stay
in natural order).

Causal mask: scores_masked = min(BIG*(q+0.5) - BIG*(k), s) applied to the
128-wide diagonal slice of diagonal key-blocks; exp(0.125*masked) == 0.
Diagonal-block score/exp/ctx work is restricted to the unmasked query range.
Softmax denominator rides the ctx matmul as an appended ones-column on V.
"""

import sys
from contextlib import ExitStack

import numpy as np
import ml_dtypes

for _p in ("/opt/trn_rl_repo",):
    if _p not in sys.path:
        sys.path.insert(0, _p)

import concourse.bass as bass
import concourse.tile as tile
from concourse import bacc, mybir
from concourse.bass_utils import run_bass_kernel_spmd

F32 = mybir.dt.float32
BF16 = mybir.dt.bfloat16
AF = mybir.ActivationFunctionType
ALU = mybir.AluOpType
BF16NP = ml_dtypes.bfloat16

B, S, D = 1, 2048, 2048
H, G, HD = 32, 8, 64
NCORES = 8
HPC = H // NCORES          # 4 q heads per core
EPS = 1e-6
BIG = 1000.0
SCW = 512                  # s-chunk width (matmul moving dim)
KBW = 128                  # key block width

# head-dim permutation: new 2j <- old j, new 2j+1 <- old j+32
PERM = np.empty(64, dtype=np.int64)
PERM[0::2] = np.arange(32)
PERM[1::2] = np.arange(32) + 32
PARTNER = np.empty(64, dtype=np.int64)
PARTNER[0::2] = np.arange(1, 64, 2)
PARTNER[1::2] = np.arange(0, 64, 2)
SIGN = np.empty(64, dtype=np.float32)
SIGN[0::2] = -1.0
SIGN[1::2] = 1.0

_SHUF_MASK = [i + 1 if i % 2 == 0 else i - 1 for i in range(32)]


def _build(nc):
    SC = S // SCW
    KB = S // KBW
    DT = D // 128

    BF_NAMES = {"xT", "wqT", "wkvT", "woT", "hsq", "hsk", "ident", "tri"}
    dt_in = {}
    for name, shape in [
        ("xT", [D, S]), ("wqT", [D, 2 * 128]), ("wkvT", [D, 128]),
        ("woT", [2 * 128, D]), ("cgq", [64, S]), ("sgq", [64, S]),
        ("cgk", [64, S]), ("sgk", [64, S]), ("hsq", [128, 2]),
        ("hsk", [64, 1]), ("bcq", [2, 128]), ("bck", [1, 64]),
        ("tri", [128, 2 * KBW]), ("ident", [64, 64]),
    ]:
        dt_in[name] = nc.dram_tensor(
            name, shape, BF16 if name in BF_NAMES else F32,
            kind="ExternalInput").ap()
    y_dram = nc.dram_tensor("y", [S, D], BF16, kind="ExternalOutput").ap()

    with tile.TileContext(nc) as tc, ExitStack() as ctx:
        ctx.enter_context(nc.allow_low_precision(
            reason="bf16 matmul operands; fp32 PSUM accumulation throughout"))
        consts = ctx.enter_context(tc.tile_pool(name="consts", bufs=1))
        persist = ctx.enter_context(tc.tile_pool(name="persist", bufs=1))
        tmp = ctx.enter_context(tc.tile_pool(name="tmp", bufs=2))

        def load(name, shape=None, double=False, eng=None):
            ap = dt_in[name]
            eng = eng or nc.sync
            shape = shape or list(ap.shape)
            t = consts.tile(shape, ap.dtype, tag=name, name=name)
            if double:
                eng.dma_start(t[0:64], ap)
                eng.dma_start(t[64:128], ap)
            else:
                eng.dma_start(t, ap.rearrange("(t p) s -> p t s", p=128)
                              if len(shape) == 3 else ap)
            return t

        # DMA priority: weights needed first (wq split so the very first
        # matmul only waits half of it), big tables after the first x chunk,
        # wo only before the first out-projection (inside the loop).
        wq_t = consts.tile([128, DT, 256], BF16, tag="wqT", name="wqT")
        wq_r = dt_in["wqT"].rearrange("(t p) s -> p t s", p=128)
        nc.sync.dma_start(wq_t[:, 0:8, :], wq_r[:, 0:8, :])
        nc.sync.dma_start(wq_t[:, 8:16, :], wq_r[:, 8:16, :])
        # everything not needed by the first matmuls goes on the scalar
        # queue so the sync queue serves the first x chunk immediately
        wkv_t = load("wkvT", [128, DT, 128], eng=nc.scalar)
        hsq_t = load("hsq", eng=nc.scalar); hsk_t = load("hsk", eng=nc.scalar)
        bcq_t = load("bcq", eng=nc.scalar); bck_t = load("bck", eng=nc.scalar)
        tri_t = consts.tile([128, 2, KBW], BF16, tag="tri", name="tri")
        nc.scalar.dma_start(tri_t, dt_in["tri"].rearrange("p (a b) -> p a b",
                                                          a=2))
        ident_t = load("ident", eng=nc.scalar)
        eps_t = consts.tile([128, 1], F32, tag="eps", name="eps")
        nc.vector.memset(eps_t, EPS)

        qrt = [persist.tile([128, S], BF16, tag=f"qrt{p}", name=f"qrt{p}")
               for p in range(2)]
        krt = persist.tile([128, S], BF16, tag="krt")
        vt = persist.tile([64, S], BF16, tag="vt")
        vaug = persist.tile([128, KB, 65], BF16, tag="vaug")
        ctxT = [persist.tile([128, S], BF16, tag=f"ctxT{p}", name=f"ctxT{p}")
                for p in range(2)]
        ones_t = consts.tile([128, KB], BF16, tag="ones", name="ones")
        nc.vector.memset(ones_t, 1.0)
        nc.vector.tensor_copy(out=vaug[:, :, 64:65],
                              in_=ones_t.rearrange("p (k o) -> p k o", o=1))

        xT_r = dt_in["xT"].rearrange("(t p) s -> p t s", p=128)

        # ---- phase 1: projections + norm/rope + V transposes ----
        # Cross-engine chains are software-pipelined one chunk deep so the
        # PE never stalls on ACT/DVE round trips:
        #   stage A (right after proj(sc) MMs): Square/shuffle/rope muls
        #   stage B (mid proj(sc+1) MMs):       rms matmuls + V transposes
        #   stage C (after proj(sc+1) MMs):     broadcast + final norm muls
        with tc.tile_pool(name="xin", bufs=2) as xin, \
             tc.tile_pool(name="projps", bufs=3, space="PSUM") as projps, \
             tc.tile_pool(name="smallps", bufs=2, space="PSUM") as smallps, \
             tc.tile_pool(name="tps", bufs=1, space="PSUM") as tpsp:

            def stage_a(sc, ps_list, st):
                # consumes the proj PSUM tiles; all ACT/DVE/GPS work
                for i, n_rows in ((0, 128), (1, 128), (2, 64)):
                    ps = ps_list[i] if i < 2 else ps_list[2][0:64, :]
                    cg = cgq_t if i < 2 else cgk_t
                    sg = sgq_t if i < 2 else sgk_t
                    # k (i=2) stays fp32 so the gpsimd-broadcast 1/rms can
                    # multiply it dtype-matched
                    rdt = BF16 if i < 2 else F32
                    sl = bass.ts(sc, SCW)
                    sq = tmp.tile([n_rows, SCW], BF16, tag=f"sq{i}", bufs=2,
                                  name=f"sq{sc}_{i}")
                    nc.scalar.activation(sq, ps, AF.Square)
                    shuf = tmp.tile([n_rows, SCW], F32, tag=f"shuf{i}", bufs=2,
                                    name=f"shuf{sc}_{i}")
                    nc.vector.stream_shuffle(shuf, ps, mask=_SHUF_MASK)
                    t1 = tmp.tile([n_rows, SCW], rdt, tag=f"t1_{i}", bufs=2,
                                  name=f"t1_{sc}_{i}")
                    nc.vector.tensor_mul(t1, ps, cg[0:n_rows, sl])
                    t2 = tmp.tile([n_rows, SCW], rdt, tag=f"t2_{i}", bufs=2,
                                  name=f"t2_{sc}_{i}")
                    nc.gpsimd.tensor_mul(t2, shuf, sg[0:n_rows, sl])
                    t3 = tmp.tile([n_rows, SCW], rdt, tag=f"t3_{i}", bufs=3,
                                  name=f"t3_{sc}_{i}")
                    nc.gpsimd.tensor_add(t3, t1, t2)
                    st[f"sq{i}"] = sq
                    st[f"t3{i}"] = t3
                nc.scalar.copy(vt[:, bass.ts(sc, SCW)], ps_list[2][64:128, :])

            def stage_b(sc, st):
                # rms reduction matmuls + sqrt + reciprocal; V transposes
                for i, (hs, nh) in enumerate(((hsq_t, 2), (hsq_t, 2),
                                              (hsk_t, 1))):
                    rms_ps = smallps.tile([nh, SCW], F32, tag="rms",
                                          name=f"rms{sc}_{i}")
                    nc.tensor.matmul(rms_ps, hs, st[f"sq{i}"],
                                     start=True, stop=True)
                    rmss = tmp.tile([nh, SCW], F32, tag="rmss", bufs=3,
                                    name=f"rmss{sc}_{i}")
                    nc.scalar.activation(rmss, rms_ps, AF.Sqrt,
                                         bias=eps_t[0:nh])
                    rcp = tmp.tile([nh, SCW], F32, tag=f"rcp{i}", bufs=3,
                                   name=f"rcp{sc}_{i}")
                    nc.vector.reciprocal_approx_fast(rcp, rmss)
                    st[f"rcp{i}"] = rcp
                for t in range(4 * sc, 4 * sc + 4):
                    tps = tpsp.tile([128, 64], BF16, tag="tps")
                    nc.tensor.transpose(tps, vt[:, bass.ts(t, KBW)], ident_t)
                    nc.vector.tensor_copy(out=vaug[:, t, 0:64], in_=tps)

            def stage_c(sc, st):
                sl = bass.ts(sc, SCW)
                # q: broadcast 1/rms via matmul (2 heads per pack)
                for p in range(2):
                    bc_ps = smallps.tile([128, SCW], F32, tag="bc",
                                         name=f"bc{sc}_{p}")
                    nc.tensor.matmul(bc_ps, bcq_t, st[f"rcp{p}"],
                                     start=True, stop=True)
                    bc_sb = tmp.tile([128, SCW], BF16, tag="bc_sb", bufs=3,
                                     name=f"bcsb{sc}_{p}")
                    nc.scalar.copy(bc_sb, bc_ps)
                    nc.vector.tensor_mul(qrt[p][:, sl], st[f"t3{p}"], bc_sb)
                # k: gpsimd broadcast (base-0 in and out), fp32 until the mul
                rck = tmp.tile([64, SCW], F32, tag="rck", bufs=3,
                               name=f"rck{sc}")
                nc.gpsimd.partition_broadcast(rck, st["rcp2"])
                nc.vector.tensor_mul(krt[0:64, sl], st["t32"], rck)
                nc.gpsimd.tensor_copy(out=krt[64:128, sl], in_=krt[0:64, sl])

            xts = {}

            def xt_dma(sc, nsplit=1):
                t = xin.tile([128, DT, SCW], BF16, tag="xt", name=f"xt{sc}")
                step = DT // nsplit
                for c in range(0, DT, step):
                    nc.sync.dma_start(t[:, c:c + step, :],
                                      xT_r[:, c:c + step, bass.ts(sc, SCW)])
                xts[sc] = t

            states = {}
            for sc in range(SC):
                if sc == 0:
                    xt_dma(0, nsplit=4)
                if sc + 1 < SC:
                    xt_dma(sc + 1)  # prefetch while this chunk computes
                xt = xts.pop(sc)
                if sc == 0:
                    # big tables via the scalar queue: parallel trigger issue,
                    # and keeps the sync queue free for the x chunks
                    cgq_t = load("cgq", [128, S], double=True, eng=nc.scalar)
                    sgq_t = load("sgq", [128, S], double=True, eng=nc.scalar)
                    cgk_t = load("cgk", eng=nc.scalar)
                    sgk_t = load("sgk", eng=nc.scalar)
                ps_list = [projps.tile([128, SCW], F32, tag="proj",
                                       name=f"proj{sc}_{i}") for i in range(3)]
                for dt_i in range(DT):
                    if dt_i == 8 and sc - 2 in states:
                        stage_c(sc - 2, states.pop(sc - 2))
                    for i, (w_t, cols) in enumerate(
                            [(wq_t, slice(0, 128)), (wq_t, slice(128, 256)),
                             (wkv_t, slice(0, 128))]):
                        nc.tensor.matmul(
                            ps_list[i], w_t[:, dt_i, cols],
                            xt[:, dt_i, :],
                            start=(dt_i == 0), stop=(dt_i == DT - 1))
                if sc == 1:
                    wo_t = load("woT", [128, 2, D], eng=nc.scalar)
                if sc - 1 in states:
                    stage_b(sc - 1, states[sc - 1])
                states[sc] = {}
                stage_a(sc, ps_list, states[sc])
            stage_c(SC - 2, states.pop(SC - 2))
            stage_b(SC - 1, states[SC - 1])
            stage_c(SC - 1, states.pop(SC - 1))

        # ---- phase 2: attention, with out-proj tiles interleaved ----
        oq = []
        evac_flip = [0]

        def emit_outproj(ypsp, youtp, n=1, force_act=False):
            for _ in range(n):
                if not oq:
                    return
                sb, dc = oq.pop(0)
                yps = ypsp.tile([128, SCW], F32, tag="yps")
                for p in range(2):
                    nc.tensor.matmul(
                        yps, ctxT[p][:, bass.ts(sb, 128)],
                        wo_t[:, p, bass.ts(dc, SCW)],
                        start=(p == 0), stop=(p == 1))
                yt = youtp.tile([128, SCW], BF16, tag="yt")
                if not force_act and evac_flip[0] % 3 != 2:
                    nc.vector.tensor_copy(out=yt, in_=yps)
                else:
                    nc.scalar.copy(yt, yps)
                evac_flip[0] += 1
                nc.sync.dma_start(
                    y_dram[bass.ts(sb, 128), bass.ts(dc, SCW)], yt)

        with tc.tile_pool(name="gps", bufs=2, space="PSUM") as gpsp, \
             tc.tile_pool(name="cps", bufs=1, space="PSUM") as cpsp, \
             tc.tile_pool(name="yps", bufs=2, space="PSUM") as ypsp, \
             tc.tile_pool(name="epool", bufs=3) as epool, \
             tc.tile_pool(name="yout", bufs=2) as youtp:
            for jc in range(SC):
                nkb = 4 * (jc + 1)
                for pack in range(2):
                    cps = cpsp.tile([65, 2, SCW], F32, tag="cps",
                                    name=f"cps{jc}_{pack}")
                    pend_ctx = None
                    for kb in range(nkb):
                        r = kb - 4 * jc
                        qlo = 128 * r if r >= 0 else 0
                        qsl = slice(qlo, SCW)
                        gsl = slice(jc * SCW + qlo, (jc + 1) * SCW)
                        gp = gpsp.tile([128, 2, SCW], F32, tag="gp",
                                       name=f"gp{jc}_{pack}_{kb}")
                        for h in range(2):
                            b = 64 * h
                            nc.tensor.matmul(
                                gp[:, h, qsl], krt[b:b + 64, bass.ts(kb, KBW)],
                                qrt[pack][b:b + 64, gsl],
                                start=True, stop=True, tile_position=(b, 0))
                        et = epool.tile([128, 2, SCW], BF16, tag="et")
                        nc.scalar.activation(et[:, :, qsl], gp[:, :, qsl],
                                             AF.Exp, scale=HD ** -0.5)
                        if r >= 0:
                            # zero the masked (key > query) triangle of the
                            # diagonal block post-exp, off the ACT chain
                            dsl = slice(qlo, qlo + KBW)
                            nc.vector.tensor_mul(et[:, :, dsl],
                                                 et[:, :, dsl], tri_t)
                        emit_outproj(ypsp, youtp)
                        if pend_ctx is not None:
                            pkb, pet, pqsl = pend_ctx
                            for h in range(2):
                                nc.tensor.matmul(
                                    cps[:, h, pqsl], vaug[:, pkb, :],
                                    pet[:, h, pqsl],
                                    start=(pkb == 0), stop=(pkb == nkb - 1))
                        pend_ctx = (kb, et, qsl)
                    pkb, pet, pqsl = pend_ctx
                    for h in range(2):
                        nc.tensor.matmul(
                            cps[:, h, pqsl], vaug[:, pkb, :], pet[:, h, pqsl],
                            start=(pkb == 0), stop=(pkb == nkb - 1))
                    # normalize: 1/denominator, broadcast on gpsimd.
                    # (custom ops need base partition 0 -> per-h staging)
                    rcps = []
                    for h in range(2):
                        dn = tmp.tile([1, SCW], F32, tag="adn", bufs=2,
                                      name=f"adn{jc}_{pack}_{h}")
                        nc.vector.tensor_copy(out=dn, in_=cps[64:65, h, :])
                        rcp = tmp.tile([1, SCW], F32, tag="arcp", bufs=2,
                                       name=f"arcp{jc}_{pack}_{h}")
                        nc.vector.reciprocal_approx_fast(rcp, dn)
                        rcps.append(rcp)
                    emit_outproj(ypsp, youtp, n=2)
                    for h in range(2):
                        bc_sb = tmp.tile([64, SCW], F32, tag="abc_sb", bufs=2,
                                         name=f"abcsb{jc}_{pack}_{h}")
                        nc.gpsimd.partition_broadcast(bc_sb, rcps[h])
                        nc.vector.tensor_mul(
                            ctxT[pack][64 * h:64 * h + 64, bass.ts(jc, SCW)],
                            cps[0:64, h, :], bc_sb)
                for sb in range(4 * jc, 4 * jc + 4):
                    for dc in range(D // SCW):
                        oq.append((sb, dc))
            emit_outproj(ypsp, youtp, n=len(oq), force_act=True)


_CACHE = {}


def _get_nc():
    if "nc" not in _CACHE:
        nc = bacc.Bacc("TRN2", target_bir_lowering=False, debug=False)
        _build(nc)
        nc.compile()
        _CACHE["nc"] = nc
    return _CACHE["nc"]


def _prep_core_inputs(c, x, cos, sin, wq, wk, wv, wo, qg, kg):
    """Host-side sharding + layout marshaling for core c."""
    f = np.float32
    hsl = slice(c * HPC * HD, (c + 1) * HPC * HD)
    gsl = slice(c * HD, (c + 1) * HD)

    wq_c = wq[hsl, :].reshape(HPC, HD, D)[:, PERM, :].reshape(HPC * HD, D)
    wk_c = wk[gsl, :][PERM, :]
    wv_c = wv[gsl, :]

    cos_p, sin_p = cos[:, PERM], sin[:, PERM]
    qg_p, kg_p = qg[PERM], kg[PERM]

    cgq = np.ascontiguousarray((cos_p * qg_p).T, dtype=f)
    sgq = np.ascontiguousarray((sin_p * (SIGN * qg_p[PARTNER])).T, dtype=f)
    cgk = np.ascontiguousarray((cos_p * kg_p).T, dtype=f)
    sgk = np.ascontiguousarray((sin_p * (SIGN * kg_p[PARTNER])).T, dtype=f)

    hsq = np.zeros((128, 2), f)
    hsq[0:64, 0] = 1.0 / HD
    hsq[64:128, 1] = 1.0 / HD
    hsk = np.full((64, 1), 1.0 / HD, f)
    bcq = np.zeros((2, 128), f)
    bcq[0, 0:64] = 1.0
    bcq[1, 64:128] = 1.0
    bck = np.ones((1, 64), f)

    # tri[p, h, q] = 1 where key p <= query q within a diagonal 128-block
    tri1 = (np.arange(128)[:, None] <= np.arange(KBW)[None, :]).astype(f)
    tri = np.concatenate([tri1, tri1], axis=1)  # [128, 2*KBW]

    bf = BF16NP
    return {
        "xT": np.ascontiguousarray(x[0].T).astype(bf),
        "wqT": np.ascontiguousarray(wq_c.T).astype(bf),
        "wkvT": np.ascontiguousarray(np.concatenate([wk_c, wv_c], 0).T).astype(bf),
        "woT": np.ascontiguousarray(wo[:, hsl].T).astype(bf),
        "cgq": cgq, "sgq": sgq, "cgk": cgk, "sgk": sgk,
        "hsq": hsq.astype(bf), "hsk": hsk.astype(bf),
        "bcq": bcq, "bck": bck,
        "tri": tri.astype(bf),
        "ident": np.eye(64, dtype=f).astype(bf),
    }


def kernel(x, mask, cos, sin, wq, wk, wv, wo, qg, kg, _trace=False):
    nc = _get_nc()
    in_maps = [
        _prep_core_inputs(c, np.asarray(x), np.asarray(cos), np.asarray(sin),
                          np.asarray(wq), np.asarray(wk), np.asarray(wv),
                          np.asarray(wo), np.asarray(qg), np.asarray(kg))
        for c in range(NCORES)
    ]
    res = run_bass_kernel_spmd(nc, in_maps, core_ids=list(range(NCORES)),
                               trace=_trace)
    _CACHE["last_results"] = res
    out = np.zeros((S, D), dtype=np.float64)
    for r in res.results:
        out += np.asarray(r["y"]).astype(np.float64)
    return out.astype(np.float32).reshape(B, S, D)


# revision 40
# speedup vs baseline: 1.0258x; 1.0258x over previous
"""GQA attention kernel for Trainium2, tensor-parallel over 8 NeuronCores.

Sharding: 4 q-heads + 1 kv-group per core (H=32, G=8). Each core computes
its heads' attention and a partial out-projection; host sums the 8 partials
(the "all-reduce after out_proj").

Layout: everything transposed (head_dim on partitions, sequence on the free
dim) so Q/K projections, scores and ctx matmuls run with 512-wide moving
operands; no runtime transposes needed except V (16 small PE transposes).

All matmul operands are bf16 (tolerance is 2e-2; fp32 runs the PE in the
4x-slower fp32_mode=HIGH path); scores/ctx/out accumulate in fp32 PSUM.

The PE stream is kept dense (HAM stays un-throttled) by software-pipelining
cross-engine chains: the RMSNorm reduction/broadcast matmuls of chunk sc are
spliced into the middle of chunk sc+1's projection matmuls, and attention
out-projection tiles are interleaved into the exp-bound attention loop.

RMSNorm over head_dim (= partitions) is a ones-matmul partition reduction +
Sqrt + reciprocal_approx_fast + broadcast (matmul for q, gpsimd
partition_broadcast for k/attention — the custom ops need base partition 0);
RoPE's rotate_half is a 32-lane pairwise stream_shuffle (head-dim components
permuted host-side into (i, i+32) pairs; wo is NOT permuted since V/ctx stay
in natural order).

Causal mask: scores_masked = min(BIG*(q+0.5) - BIG*(k), s) applied to the
128-wide diagonal slice of diagonal key-blocks; exp(0.125*masked) == 0.
Diagonal-block score/exp/ctx work is restricted to the unmasked query range.
Softmax denominator rides the ctx matmul as an appended ones-column on V.
"""

import sys
from contextlib import ExitStack

import numpy as np
import ml_dtypes

for _p in ("/opt/trn_rl_repo",):
    if _p not in sys.path:
        sys.path.insert(0, _p)

import concourse.bass as bass
import concourse.tile as tile
from concourse import bacc, mybir
from concourse.bass_utils import run_bass_kernel_spmd

F32 = mybir.dt.float32
BF16 = mybir.dt.bfloat16
AF = mybir.ActivationFunctionType
ALU = mybir.AluOpType
BF16NP = ml_dtypes.bfloat16

B, S, D = 1, 2048, 2048
H, G, HD = 32, 8, 64
NCORES = 8
HPC = H // NCORES          # 4 q heads per core
EPS = 1e-6
BIG = 1000.0
SCW = 512                  # s-chunk width (matmul moving dim)
KBW = 128                  # key block width

# head-dim permutation: new 2j <- old j, new 2j+1 <- old j+32
PERM = np.empty(64, dtype=np.int64)
PERM[0::2] = np.arange(32)
PERM[1::2] = np.arange(32) + 32
PARTNER = np.empty(64, dtype=np.int64)
PARTNER[0::2] = np.arange(1, 64, 2)
PARTNER[1::2] = np.arange(0, 64, 2)
SIGN = np.empty(64, dtype=np.float32)
SIGN[0::2] = -1.0
SIGN[1::2] = 1.0

_SHUF_MASK = [i + 1 if i % 2 == 0 else i - 1 for i in range(32)]


def _build(nc):
    SC = S // SCW
    KB = S // KBW
    DT = D // 128

    BF_NAMES = {"xT", "wqT", "wkvT", "woT", "hsq", "hsk", "ident", "tri"}
    dt_in = {}
    for name, shape in [
        ("xT", [D, S]), ("wqT", [D, 2 * 128]), ("wkvT", [D, 128]),
        ("woT", [2 * 128, D]), ("cgq", [64, S]), ("sgq", [64, S]),
        ("cgk", [64, S]), ("sgk", [64, S]), ("hsq", [128, 2]),
        ("hsk", [64, 1]), ("bcq", [2, 128]), ("bck", [1, 64]),
        ("tri", [128, 2 * KBW]), ("ident", [64, 64]),
    ]:
        dt_in[name] = nc.dram_tensor(
            name, shape, BF16 if name in BF_NAMES else F32,
            kind="ExternalInput").ap()
    y_dram = nc.dram_tensor("y", [S, D], BF16, kind="ExternalOutput").ap()

    with tile.TileContext(nc) as tc, ExitStack() as ctx:
        ctx.enter_context(nc.allow_low_precision(
            reason="bf16 matmul operands; fp32 PSUM accumulation throughout"))
        consts = ctx.enter_context(tc.tile_pool(name="consts", bufs=1))
        persist = ctx.enter_context(tc.tile_pool(name="persist", bufs=1))
        tmp = ctx.enter_context(tc.tile_pool(name="tmp", bufs=2))

        def load(name, shape=None, double=False, eng=None):
            ap = dt_in[name]
            eng = eng or nc.sync
            shape = shape or list(ap.shape)
            t = consts.tile(shape, ap.dtype, tag=name, name=name)
            if double:
                eng.dma_start(t[0:64], ap)
                eng.dma_start(t[64:128], ap)
            else:
                eng.dma_start(t, ap.rearrange("(t p) s -> p t s", p=128)
                              if len(shape) == 3 else ap)
            return t

        # DMA priority: weights needed first (wq split so the very first
        # matmul only waits half of it), big tables after the first x chunk,
        # wo only before the first out-projection (inside the loop).
        wq_t = consts.tile([128, DT, 256], BF16, tag="wqT", name="wqT")
        wq_r = dt_in["wqT"].rearrange("(t p) s -> p t s", p=128)
        nc.sync.dma_start(wq_t[:, 0:8, :], wq_r[:, 0:8, :])
        nc.sync.dma_start(wq_t[:, 8:16, :], wq_r[:, 8:16, :])
        # everything not needed by the first matmuls goes on the scalar
        # queue so the sync queue serves the first x chunk immediately
        wkv_t = load("wkvT", [128, DT, 128], eng=nc.scalar)
        hsq_t = load("hsq", eng=nc.scalar); hsk_t = load("hsk", eng=nc.scalar)
        bcq_t = load("bcq", eng=nc.scalar); bck_t = load("bck", eng=nc.scalar)
        tri_t = consts.tile([128, 2, KBW], BF16, tag="tri", name="tri")
        nc.scalar.dma_start(tri_t, dt_in["tri"].rearrange("p (a b) -> p a b",
                                                          a=2))
        ident_t = load("ident", eng=nc.scalar)
        eps_t = consts.tile([128, 1], F32, tag="eps", name="eps")
        nc.vector.memset(eps_t, EPS)

        qrt = [persist.tile([128, S], BF16, tag=f"qrt{p}", name=f"qrt{p}")
               for p in range(2)]
        krt = persist.tile([128, S], BF16, tag="krt")
        vt = persist.tile([64, S], BF16, tag="vt")
        vaug = persist.tile([128, KB, 65], BF16, tag="vaug")
        ctxT = [persist.tile([128, S], BF16, tag=f"ctxT{p}", name=f"ctxT{p}")
                for p in range(2)]
        ones_t = consts.tile([128, KB], BF16, tag="ones", name="ones")
        nc.vector.memset(ones_t, 1.0)
        nc.vector.tensor_copy(out=vaug[:, :, 64:65],
                              in_=ones_t.rearrange("p (k o) -> p k o", o=1))

        xT_r = dt_in["xT"].rearrange("(t p) s -> p t s", p=128)

        # ---- phase 1: projections + norm/rope + V transposes ----
        # Cross-engine chains are software-pipelined one chunk deep so the
        # PE never stalls on ACT/DVE round trips:
        #   stage A (right after proj(sc) MMs): Square/shuffle/rope muls
        #   stage B (mid proj(sc+1) MMs):       rms matmuls + V transposes
        #   stage C (after proj(sc+1) MMs):     broadcast + final norm muls
        with tc.tile_pool(name="xin", bufs=4) as xin, \
             tc.tile_pool(name="projps", bufs=3, space="PSUM") as projps, \
             tc.tile_pool(name="smallps", bufs=2, space="PSUM") as smallps, \
             tc.tile_pool(name="tps", bufs=1, space="PSUM") as tpsp:

            def stage_a(sc, ps_list, st):
                # consumes the proj PSUM tiles; all ACT/DVE/GPS work
                for i, n_rows in ((0, 128), (1, 128), (2, 64)):
                    ps = ps_list[i] if i < 2 else ps_list[2][0:64, :]
                    cg = cgq_t if i < 2 else cgk_t
                    sg = sgq_t if i < 2 else sgk_t
                    # k (i=2) stays fp32 so the gpsimd-broadcast 1/rms can
                    # multiply it dtype-matched
                    rdt = BF16 if i < 2 else F32
                    sl = bass.ts(sc, SCW)
                    sq = tmp.tile([n_rows, SCW], BF16, tag=f"sq{i}", bufs=2,
                                  name=f"sq{sc}_{i}")
                    nc.scalar.activation(sq, ps, AF.Square)
                    shuf = tmp.tile([n_rows, SCW], F32, tag=f"shuf{i}", bufs=2,
                                    name=f"shuf{sc}_{i}")
                    nc.vector.stream_shuffle(shuf, ps, mask=_SHUF_MASK)
                    t1 = tmp.tile([n_rows, SCW], rdt, tag=f"t1_{i}", bufs=2,
                                  name=f"t1_{sc}_{i}")
                    nc.vector.tensor_mul(t1, ps, cg[0:n_rows, sl])
                    t2 = tmp.tile([n_rows, SCW], rdt, tag=f"t2_{i}", bufs=2,
                                  name=f"t2_{sc}_{i}")
                    nc.gpsimd.tensor_mul(t2, shuf, sg[0:n_rows, sl])
                    t3 = tmp.tile([n_rows, SCW], rdt, tag=f"t3_{i}", bufs=3,
                                  name=f"t3_{sc}_{i}")
                    nc.gpsimd.tensor_add(t3, t1, t2)
                    st[f"sq{i}"] = sq
                    st[f"t3{i}"] = t3
                nc.scalar.copy(vt[:, bass.ts(sc, SCW)], ps_list[2][64:128, :])

            def stage_b(sc, st):
                # rms reduction matmuls + sqrt + reciprocal; V transposes
                for i, (hs, nh) in enumerate(((hsq_t, 2), (hsq_t, 2),
                                              (hsk_t, 1))):
                    rms_ps = smallps.tile([nh, SCW], F32, tag="rms",
                                          name=f"rms{sc}_{i}")
                    nc.tensor.matmul(rms_ps, hs, st[f"sq{i}"],
                                     start=True, stop=True)
                    rmss = tmp.tile([nh, SCW], F32, tag="rmss", bufs=3,
                                    name=f"rmss{sc}_{i}")
                    nc.scalar.activation(rmss, rms_ps, AF.Sqrt,
                                         bias=eps_t[0:nh])
                    rcp = tmp.tile([nh, SCW], F32, tag=f"rcp{i}", bufs=3,
                                   name=f"rcp{sc}_{i}")
                    nc.vector.reciprocal_approx_fast(rcp, rmss)
                    st[f"rcp{i}"] = rcp
                for t in range(4 * sc, 4 * sc + 4):
                    tps = tpsp.tile([128, 64], BF16, tag="tps")
                    nc.tensor.transpose(tps, vt[:, bass.ts(t, KBW)], ident_t)
                    nc.vector.tensor_copy(out=vaug[:, t, 0:64], in_=tps)

            def stage_c(sc, st):
                sl = bass.ts(sc, SCW)
                # q: broadcast 1/rms via matmul (2 heads per pack)
                for p in range(2):
                    bc_ps = smallps.tile([128, SCW], F32, tag="bc",
                                         name=f"bc{sc}_{p}")
                    nc.tensor.matmul(bc_ps, bcq_t, st[f"rcp{p}"],
                                     start=True, stop=True)
                    bc_sb = tmp.tile([128, SCW], BF16, tag="bc_sb", bufs=3,
                                     name=f"bcsb{sc}_{p}")
                    nc.scalar.copy(bc_sb, bc_ps)
                    nc.vector.tensor_mul(qrt[p][:, sl], st[f"t3{p}"], bc_sb)
                # k: gpsimd broadcast (base-0 in and out), fp32 until the mul
                rck = tmp.tile([64, SCW], F32, tag="rck", bufs=3,
                               name=f"rck{sc}")
                nc.gpsimd.partition_broadcast(rck, st["rcp2"])
                nc.vector.tensor_mul(krt[0:64, sl], st["t32"], rck)
                nc.gpsimd.tensor_copy(out=krt[64:128, sl], in_=krt[0:64, sl])

            xts = {}
            HDT = DT // 2

            def xt_dma(sc, half):
                t = xin.tile([128, HDT, SCW], BF16, tag="xt",
                             name=f"xt{sc}_{half}")
                nc.sync.dma_start(
                    t, xT_r[:, half * HDT:(half + 1) * HDT, bass.ts(sc, SCW)])
                xts[(sc, half)] = t

            states = {}
            for sc in range(SC):
                if sc == 0:
                    for key in ((0, 0), (0, 1), (1, 0), (1, 1)):
                        xt_dma(*key)
                elif sc + 1 < SC:
                    xt_dma(sc + 1, 0)
                    xt_dma(sc + 1, 1)
                if sc == 0:
                    # big tables via the scalar queue: parallel trigger issue,
                    # and keeps the sync queue free for the x chunks
                    cgq_t = load("cgq", [128, S], double=True, eng=nc.scalar)
                    sgq_t = load("sgq", [128, S], double=True, eng=nc.scalar)
                    cgk_t = load("cgk", eng=nc.scalar)
                    sgk_t = load("sgk", eng=nc.scalar)
                ps_list = [projps.tile([128, SCW], F32, tag="proj",
                                       name=f"proj{sc}_{i}") for i in range(3)]
                for dt_i in range(DT):
                    if dt_i == 8 and sc - 2 in states:
                        stage_c(sc - 2, states.pop(sc - 2))
                    xt = xts[(sc, dt_i // HDT)]
                    for i, (w_t, cols) in enumerate(
                            [(wq_t, slice(0, 128)), (wq_t, slice(128, 256)),
                             (wkv_t, slice(0, 128))]):
                        nc.tensor.matmul(
                            ps_list[i], w_t[:, dt_i, cols],
                            xt[:, dt_i % HDT, :],
                            start=(dt_i == 0), stop=(dt_i == DT - 1))
                del xts[(sc, 0)], xts[(sc, 1)]
                if sc == 1:
                    wo_t = load("woT", [128, 2, D], eng=nc.scalar)
                if sc - 1 in states:
                    stage_b(sc - 1, states[sc - 1])
                states[sc] = {}
                stage_a(sc, ps_list, states[sc])
            stage_c(SC - 2, states.pop(SC - 2))
            stage_b(SC - 1, states[SC - 1])
            stage_c(SC - 1, states.pop(SC - 1))

        # ---- phase 2: attention, with out-proj tiles interleaved ----
        oq = []
        evac_flip = [0]

        def emit_outproj(ypsp, youtp, n=1, force_act=False):
            for _ in range(n):
                if not oq:
                    return
                sb, dc = oq.pop(0)
                yps = ypsp.tile([128, SCW], F32, tag="yps")
                for p in range(2):
                    nc.tensor.matmul(
                        yps, ctxT[p][:, bass.ts(sb, 128)],
                        wo_t[:, p, bass.ts(dc, SCW)],
                        start=(p == 0), stop=(p == 1))
                yt = youtp.tile([128, SCW], BF16, tag="yt")
                if not force_act and evac_flip[0] % 3 != 2:
                    nc.vector.tensor_copy(out=yt, in_=yps)
                else:
                    nc.scalar.copy(yt, yps)
                evac_flip[0] += 1
                nc.sync.dma_start(
                    y_dram[bass.ts(sb, 128), bass.ts(dc, SCW)], yt)

        with tc.tile_pool(name="gps", bufs=2, space="PSUM") as gpsp, \
             tc.tile_pool(name="cps", bufs=1, space="PSUM") as cpsp, \
             tc.tile_pool(name="yps", bufs=2, space="PSUM") as ypsp, \
             tc.tile_pool(name="epool", bufs=3) as epool, \
             tc.tile_pool(name="yout", bufs=2) as youtp:
            for jc in range(SC):
                nkb = 4 * (jc + 1)
                for pack in range(2):
                    cps = cpsp.tile([65, 2, SCW], F32, tag="cps",
                                    name=f"cps{jc}_{pack}")
                    pend_ctx = None
                    for kb in range(nkb):
                        r = kb - 4 * jc
                        qlo = 128 * r if r >= 0 else 0
                        qsl = slice(qlo, SCW)
                        gsl = slice(jc * SCW + qlo, (jc + 1) * SCW)
                        gp = gpsp.tile([128, 2, SCW], F32, tag="gp",
                                       name=f"gp{jc}_{pack}_{kb}")
                        for h in range(2):
                            b = 64 * h
                            nc.tensor.matmul(
                                gp[:, h, qsl], krt[b:b + 64, bass.ts(kb, KBW)],
                                qrt[pack][b:b + 64, gsl],
                                start=True, stop=True, tile_position=(b, 0))
                        et = epool.tile([128, 2, SCW], BF16, tag="et")
                        nc.scalar.activation(et[:, :, qsl], gp[:, :, qsl],
                                             AF.Exp, scale=HD ** -0.5)
                        if r >= 0:
                            # zero the masked (key > query) triangle of the
                            # diagonal block post-exp, off the ACT chain
                            dsl = slice(qlo, qlo + KBW)
                            nc.vector.tensor_mul(et[:, :, dsl],
                                                 et[:, :, dsl], tri_t)
                        emit_outproj(ypsp, youtp)
                        if pend_ctx is not None:
                            pkb, pet, pqsl = pend_ctx
                            for h in range(2):
                                nc.tensor.matmul(
                                    cps[:, h, pqsl], vaug[:, pkb, :],
                                    pet[:, h, pqsl],
                                    start=(pkb == 0), stop=(pkb == nkb - 1))
                        pend_ctx = (kb, et, qsl)
                    pkb, pet, pqsl = pend_ctx
                    for h in range(2):
                        nc.tensor.matmul(
                            cps[:, h, pqsl], vaug[:, pkb, :], pet[:, h, pqsl],
                            start=(pkb == 0), stop=(pkb == nkb - 1))
                    # normalize: 1/denominator, broadcast on gpsimd.
                    # (custom ops need base partition 0 -> per-h staging)
                    rcps = []
                    for h in range(2):
                        dn = tmp.tile([1, SCW], F32, tag="adn", bufs=2,
                                      name=f"adn{jc}_{pack}_{h}")
                        nc.vector.tensor_copy(out=dn, in_=cps[64:65, h, :])
                        rcp = tmp.tile([1, SCW], F32, tag="arcp", bufs=2,
                                       name=f"arcp{jc}_{pack}_{h}")
                        nc.vector.reciprocal_approx_fast(rcp, dn)
                        rcps.append(rcp)
                    emit_outproj(ypsp, youtp, n=2)
                    for h in range(2):
                        bc_sb = tmp.tile([64, SCW], F32, tag="abc_sb", bufs=2,
                                         name=f"abcsb{jc}_{pack}_{h}")
                        nc.gpsimd.partition_broadcast(bc_sb, rcps[h])
                        nc.vector.tensor_mul(
                            ctxT[pack][64 * h:64 * h + 64, bass.ts(jc, SCW)],
                            cps[0:64, h, :], bc_sb)
                for sb in range(4 * jc, 4 * jc + 4):
                    for dc in range(D // SCW):
                        oq.append((sb, dc))
            emit_outproj(ypsp, youtp, n=len(oq), force_act=True)


_CACHE = {}


def _get_nc():
    if "nc" not in _CACHE:
        nc = bacc.Bacc("TRN2", target_bir_lowering=False, debug=False)
        _build(nc)
        nc.compile()
        _CACHE["nc"] = nc
    return _CACHE["nc"]


def _prep_core_inputs(c, x, cos, sin, wq, wk, wv, wo, qg, kg):
    """Host-side sharding + layout marshaling for core c."""
    f = np.float32
    hsl = slice(c * HPC * HD, (c + 1) * HPC * HD)
    gsl = slice(c * HD, (c + 1) * HD)

    wq_c = wq[hsl, :].reshape(HPC, HD, D)[:, PERM, :].reshape(HPC * HD, D)
    wk_c = wk[gsl, :][PERM, :]
    wv_c = wv[gsl, :]

    cos_p, sin_p = cos[:, PERM], sin[:, PERM]
    qg_p, kg_p = qg[PERM], kg[PERM]

    cgq = np.ascontiguousarray((cos_p * qg_p).T, dtype=f)
    sgq = np.ascontiguousarray((sin_p * (SIGN * qg_p[PARTNER])).T, dtype=f)
    cgk = np.ascontiguousarray((cos_p * kg_p).T, dtype=f)
    sgk = np.ascontiguousarray((sin_p * (SIGN * kg_p[PARTNER])).T, dtype=f)

    hsq = np.zeros((128, 2), f)
    hsq[0:64, 0] = 1.0 / HD
    hsq[64:128, 1] = 1.0 / HD
    hsk = np.full((64, 1), 1.0 / HD, f)
    bcq = np.zeros((2, 128), f)
    bcq[0, 0:64] = 1.0
    bcq[1, 64:128] = 1.0
    bck = np.ones((1, 64), f)

    # tri[p, h, q] = 1 where key p <= query q within a diagonal 128-block
    tri1 = (np.arange(128)[:, None] <= np.arange(KBW)[None, :]).astype(f)
    tri = np.concatenate([tri1, tri1], axis=1)  # [128, 2*KBW]

    bf = BF16NP
    return {
        "xT": np.ascontiguousarray(x[0].T).astype(bf),
        "wqT": np.ascontiguousarray(wq_c.T).astype(bf),
        "wkvT": np.ascontiguousarray(np.concatenate([wk_c, wv_c], 0).T).astype(bf),
        "woT": np.ascontiguousarray(wo[:, hsl].T).astype(bf),
        "cgq": cgq, "sgq": sgq, "cgk": cgk, "sgk": sgk,
        "hsq": hsq.astype(bf), "hsk": hsk.astype(bf),
        "bcq": bcq, "bck": bck,
        "tri": tri.astype(bf),
        "ident": np.eye(64, dtype=f).astype(bf),
    }


def kernel(x, mask, cos, sin, wq, wk, wv, wo, qg, kg, _trace=False):
    nc = _get_nc()
    in_maps = [
        _prep_core_inputs(c, np.asarray(x), np.asarray(cos), np.asarray(sin),
                          np.asarray(wq), np.asarray(wk), np.asarray(wv),
                          np.asarray(wo), np.asarray(qg), np.asarray(kg))
        for c in range(NCORES)
    ]
    res = run_bass_kernel_spmd(nc, in_maps, core_ids=list(range(NCORES)),
                               trace=_trace)
    _CACHE["last_results"] = res
    out = np.zeros((S, D), dtype=np.float64)
    for r in res.results:
        out += np.asarray(r["y"]).astype(np.float64)
    return out.astype(np.float32).reshape(B, S, D)


# revision 43
# speedup vs baseline: 1.0398x; 1.0136x over previous
"""GQA attention kernel for Trainium2, tensor-parallel over 8 NeuronCores.

Sharding: 4 q-heads + 1 kv-group per core (H=32, G=8). Each core computes
its heads' attention and a partial out-projection; host sums the 8 partials
(the "all-reduce after out_proj").

Layout: everything transposed (head_dim on partitions, sequence on the free
dim) so Q/K projections, scores and ctx matmuls run with 512-wide moving
operands; no runtime transposes needed except V (16 small PE transposes).

All matmul operands are bf16 (tolerance is 2e-2; fp32 runs the PE in the
4x-slower fp32_mode=HIGH path); scores/ctx/out accumulate in fp32 PSUM.

The PE stream is kept dense (HAM stays un-throttled) by software-pipelining
cross-engine chains: the RMSNorm reduction/broadcast matmuls of chunk sc are
spliced into the middle of chunk sc+1's projection matmuls, and attention
out-projection tiles are interleaved into the exp-bound attention loop.

RMSNorm over head_dim (= partitions) is a ones-matmul partition reduction +
Sqrt + reciprocal_approx_fast + broadcast (matmul for q, gpsimd
partition_broadcast for k/attention — the custom ops need base partition 0);
RoPE's rotate_half is a 32-lane pairwise stream_shuffle (head-dim components
permuted host-side into (i, i+32) pairs; wo is NOT permuted since V/ctx stay
in natural order).

Causal mask: scores_masked = min(BIG*(q+0.5) - BIG*(k), s) applied to the
128-wide diagonal slice of diagonal key-blocks; exp(0.125*masked) == 0.
Diagonal-block score/exp/ctx work is restricted to the unmasked query range.
Softmax denominator rides the ctx matmul as an appended ones-column on V.
"""

import sys
from contextlib import ExitStack

import numpy as np
import ml_dtypes

for _p in ("/opt/trn_rl_repo",):
    if _p not in sys.path:
        sys.path.insert(0, _p)

import concourse.bass as bass
import concourse.tile as tile
from concourse import bacc, mybir
from concourse.bass_utils import run_bass_kernel_spmd

F32 = mybir.dt.float32
BF16 = mybir.dt.bfloat16
AF = mybir.ActivationFunctionType
ALU = mybir.AluOpType
BF16NP = ml_dtypes.bfloat16

B, S, D = 1, 2048, 2048
H, G, HD = 32, 8, 64
NCORES = 8
HPC = H // NCORES          # 4 q heads per core
EPS = 1e-6
BIG = 1000.0
SCW = 512                  # s-chunk width (matmul moving dim)
KBW = 128                  # key block width

# head-dim permutation: new 2j <- old j, new 2j+1 <- old j+32
PERM = np.empty(64, dtype=np.int64)
PERM[0::2] = np.arange(32)
PERM[1::2] = np.arange(32) + 32
PARTNER = np.empty(64, dtype=np.int64)
PARTNER[0::2] = np.arange(1, 64, 2)
PARTNER[1::2] = np.arange(0, 64, 2)
SIGN = np.empty(64, dtype=np.float32)
SIGN[0::2] = -1.0
SIGN[1::2] = 1.0

_SHUF_MASK = [i + 1 if i % 2 == 0 else i - 1 for i in range(32)]


def _build(nc):
    SC = S // SCW
    KB = S // KBW
    DT = D // 128

    BF_NAMES = {"xT", "wqT", "wkvT", "woT", "hsq", "hsk", "ident", "tri"}
    dt_in = {}
    for name, shape in [
        ("xT", [D, S]), ("wqT", [D, 2 * 128]), ("wkvT", [D, 128]),
        ("woT", [2 * 128, D]), ("cgq", [64, S]), ("sgq", [64, S]),
        ("cgk", [64, S]), ("sgk", [64, S]), ("hsq", [128, 2]),
        ("hsk", [64, 1]), ("bcq", [2, 128]), ("bck", [1, 64]),
        ("tri", [128, 2 * KBW]), ("ident", [64, 64]),
    ]:
        dt_in[name] = nc.dram_tensor(
            name, shape, BF16 if name in BF_NAMES else F32,
            kind="ExternalInput").ap()
    y_dram = nc.dram_tensor("y", [S, D], BF16, kind="ExternalOutput").ap()

    with tile.TileContext(nc) as tc, ExitStack() as ctx:
        ctx.enter_context(nc.allow_low_precision(
            reason="bf16 matmul operands; fp32 PSUM accumulation throughout"))
        consts = ctx.enter_context(tc.tile_pool(name="consts", bufs=1))
        persist = ctx.enter_context(tc.tile_pool(name="persist", bufs=1))
        tmp = ctx.enter_context(tc.tile_pool(name="tmp", bufs=2))

        def load(name, shape=None, double=False, eng=None):
            ap = dt_in[name]
            eng = eng or nc.sync
            shape = shape or list(ap.shape)
            t = consts.tile(shape, ap.dtype, tag=name, name=name)
            if double:
                eng.dma_start(t[0:64], ap)
                eng.dma_start(t[64:128], ap)
            else:
                eng.dma_start(t, ap.rearrange("(t p) s -> p t s", p=128)
                              if len(shape) == 3 else ap)
            return t

        # DMA priority: weights needed first (wq split so the very first
        # matmul only waits half of it), big tables after the first x chunk,
        # wo only before the first out-projection (inside the loop).
        # wq on the scalar queue so it transfers concurrently with the first
        # x chunk on the sync queue
        wq_t = consts.tile([128, DT, 256], BF16, tag="wqT", name="wqT")
        wq_r = dt_in["wqT"].rearrange("(t p) s -> p t s", p=128)
        nc.scalar.dma_start(wq_t[:, 0:8, :], wq_r[:, 0:8, :])
        nc.scalar.dma_start(wq_t[:, 8:16, :], wq_r[:, 8:16, :])
        # everything not needed by the first matmuls goes on the scalar
        # queue so the sync queue serves the first x chunk immediately
        wkv_t = load("wkvT", [128, DT, 128], eng=nc.scalar)
        hsq_t = load("hsq", eng=nc.scalar); hsk_t = load("hsk", eng=nc.scalar)
        bcq_t = load("bcq", eng=nc.scalar); bck_t = load("bck", eng=nc.scalar)
        tri_t = consts.tile([128, 2, KBW], BF16, tag="tri", name="tri")
        nc.scalar.dma_start(tri_t, dt_in["tri"].rearrange("p (a b) -> p a b",
                                                          a=2))
        ident_t = load("ident", eng=nc.scalar)
        eps_t = consts.tile([128, 1], F32, tag="eps", name="eps")
        nc.vector.memset(eps_t, EPS)

        qrt = [persist.tile([128, S], BF16, tag=f"qrt{p}", name=f"qrt{p}")
               for p in range(2)]
        krt = persist.tile([128, S], BF16, tag="krt")
        vt = persist.tile([64, S], BF16, tag="vt")
        vaug = persist.tile([128, KB, 65], BF16, tag="vaug")
        ctxT = [persist.tile([128, S], BF16, tag=f"ctxT{p}", name=f"ctxT{p}")
                for p in range(2)]
        ones_t = consts.tile([128, KB], BF16, tag="ones", name="ones")
        nc.vector.memset(ones_t, 1.0)
        nc.vector.tensor_copy(out=vaug[:, :, 64:65],
                              in_=ones_t.rearrange("p (k o) -> p k o", o=1))

        xT_r = dt_in["xT"].rearrange("(t p) s -> p t s", p=128)

        # ---- phase 1: projections + norm/rope + V transposes ----
        # Cross-engine chains are software-pipelined one chunk deep so the
        # PE never stalls on ACT/DVE round trips:
        #   stage A (right after proj(sc) MMs): Square/shuffle/rope muls
        #   stage B (mid proj(sc+1) MMs):       rms matmuls + V transposes
        #   stage C (after proj(sc+1) MMs):     broadcast + final norm muls
        with tc.tile_pool(name="xin", bufs=4) as xin, \
             tc.tile_pool(name="projps", bufs=3, space="PSUM") as projps, \
             tc.tile_pool(name="smallps", bufs=2, space="PSUM") as smallps, \
             tc.tile_pool(name="tps", bufs=1, space="PSUM") as tpsp:

            def stage_a(sc, ps_list, st):
                # consumes the proj PSUM tiles; all ACT/DVE/GPS work
                for i, n_rows in ((0, 128), (1, 128), (2, 64)):
                    ps = ps_list[i] if i < 2 else ps_list[2][0:64, :]
                    cg = cgq_t if i < 2 else cgk_t
                    sg = sgq_t if i < 2 else sgk_t
                    # k (i=2) stays fp32 so the gpsimd-broadcast 1/rms can
                    # multiply it dtype-matched
                    rdt = BF16 if i < 2 else F32
                    sl = bass.ts(sc, SCW)
                    sq = tmp.tile([n_rows, SCW], BF16, tag=f"sq{i}", bufs=2,
                                  name=f"sq{sc}_{i}")
                    nc.scalar.activation(sq, ps, AF.Square)
                    shuf = tmp.tile([n_rows, SCW], F32, tag=f"shuf{i}", bufs=2,
                                    name=f"shuf{sc}_{i}")
                    nc.vector.stream_shuffle(shuf, ps, mask=_SHUF_MASK)
                    t1 = tmp.tile([n_rows, SCW], rdt, tag=f"t1_{i}", bufs=2,
                                  name=f"t1_{sc}_{i}")
                    nc.vector.tensor_mul(t1, ps, cg[0:n_rows, sl])
                    t2 = tmp.tile([n_rows, SCW], rdt, tag=f"t2_{i}", bufs=2,
                                  name=f"t2_{sc}_{i}")
                    nc.gpsimd.tensor_mul(t2, shuf, sg[0:n_rows, sl])
                    t3 = tmp.tile([n_rows, SCW], rdt, tag=f"t3_{i}", bufs=3,
                                  name=f"t3_{sc}_{i}")
                    nc.gpsimd.tensor_add(t3, t1, t2)
                    st[f"sq{i}"] = sq
                    st[f"t3{i}"] = t3
                nc.scalar.copy(vt[:, bass.ts(sc, SCW)], ps_list[2][64:128, :])

            def stage_b1(sc, st):
                # PE only: rms reduction matmuls + V transposes
                for i, (hs, nh) in enumerate(((hsq_t, 2), (hsq_t, 2),
                                              (hsk_t, 1))):
                    rms_ps = smallps.tile([nh, SCW], F32, tag="rms", bufs=3,
                                          name=f"rms{sc}_{i}")
                    nc.tensor.matmul(rms_ps, hs, st[f"sq{i}"],
                                     start=True, stop=True)
                    st[f"rms{i}"] = rms_ps
                for t in range(4 * sc, 4 * sc + 4):
                    tps = tpsp.tile([128, 64], BF16, tag="tps")
                    nc.tensor.transpose(tps, vt[:, bass.ts(t, KBW)], ident_t)
                    nc.vector.tensor_copy(out=vaug[:, t, 0:64], in_=tps)

            def stage_b2(sc, st):
                # ACT/DVE tail of the rms chain — emitted after stage_a(sc+1)
                # so the FIFO ACT queue isn't head-of-line blocked on the
                # rms matmuls
                for i, nh in ((0, 2), (1, 2), (2, 1)):
                    rmss = tmp.tile([nh, SCW], F32, tag="rmss", bufs=3,
                                    name=f"rmss{sc}_{i}")
                    nc.scalar.activation(rmss, st[f"rms{i}"], AF.Sqrt,
                                         bias=eps_t[0:nh])
                    rcp = tmp.tile([nh, SCW], F32, tag=f"rcp{i}", bufs=3,
                                   name=f"rcp{sc}_{i}")
                    nc.vector.reciprocal_approx_fast(rcp, rmss)
                    st[f"rcp{i}"] = rcp

            def stage_c(sc, st):
                sl = bass.ts(sc, SCW)
                # q: broadcast 1/rms via matmul (2 heads per pack)
                for p in range(2):
                    bc_ps = smallps.tile([128, SCW], F32, tag="bc", bufs=1,
                                         name=f"bc{sc}_{p}")
                    nc.tensor.matmul(bc_ps, bcq_t, st[f"rcp{p}"],
                                     start=True, stop=True)
                    bc_sb = tmp.tile([128, SCW], BF16, tag="bc_sb", bufs=3,
                                     name=f"bcsb{sc}_{p}")
                    nc.scalar.copy(bc_sb, bc_ps)
                    nc.vector.tensor_mul(qrt[p][:, sl], st[f"t3{p}"], bc_sb)
                # k: gpsimd broadcast (base-0 in and out), fp32 until the mul
                rck = tmp.tile([64, SCW], F32, tag="rck", bufs=3,
                               name=f"rck{sc}")
                nc.gpsimd.partition_broadcast(rck, st["rcp2"])
                nc.vector.tensor_mul(krt[0:64, sl], st["t32"], rck)
                nc.gpsimd.tensor_copy(out=krt[64:128, sl], in_=krt[0:64, sl])

            xts = {}
            HDT = DT // 2

            def xt_dma(sc, half):
                t = xin.tile([128, HDT, SCW], BF16, tag="xt",
                             name=f"xt{sc}_{half}")
                nc.sync.dma_start(
                    t, xT_r[:, half * HDT:(half + 1) * HDT, bass.ts(sc, SCW)])
                xts[(sc, half)] = t

            states = {}
            for sc in range(SC):
                if sc == 0:
                    for key in ((0, 0), (0, 1), (1, 0), (1, 1)):
                        xt_dma(*key)
                elif sc + 1 < SC:
                    xt_dma(sc + 1, 0)
                    xt_dma(sc + 1, 1)
                if sc == 0:
                    # big tables via the scalar queue: parallel trigger issue,
                    # and keeps the sync queue free for the x chunks
                    cgq_t = load("cgq", [128, S], double=True, eng=nc.scalar)
                    sgq_t = load("sgq", [128, S], double=True, eng=nc.scalar)
                    cgk_t = load("cgk", eng=nc.scalar)
                    sgk_t = load("sgk", eng=nc.scalar)
                ps_list = [projps.tile([128, SCW], F32, tag="proj",
                                       name=f"proj{sc}_{i}") for i in range(3)]
                for dt_i in range(DT):
                    if dt_i == 8 and sc - 2 in states:
                        stage_c(sc - 2, states.pop(sc - 2))
                    xt = xts[(sc, dt_i // HDT)]
                    for i, (w_t, cols) in enumerate(
                            [(wq_t, slice(0, 128)), (wq_t, slice(128, 256)),
                             (wkv_t, slice(0, 128))]):
                        nc.tensor.matmul(
                            ps_list[i], w_t[:, dt_i, cols],
                            xt[:, dt_i % HDT, :],
                            start=(dt_i == 0), stop=(dt_i == DT - 1))
                del xts[(sc, 0)], xts[(sc, 1)]
                if sc == 1:
                    wo_t = load("woT", [128, 2, D], eng=nc.scalar)
                if sc - 1 in states:
                    stage_b1(sc - 1, states[sc - 1])
                states[sc] = {}
                stage_a(sc, ps_list, states[sc])
                if sc - 1 in states:
                    stage_b2(sc - 1, states[sc - 1])
            stage_c(SC - 2, states.pop(SC - 2))
            stage_b1(SC - 1, states[SC - 1])
            stage_b2(SC - 1, states[SC - 1])
            stage_c(SC - 1, states.pop(SC - 1))

        # ---- phase 2: attention, with out-proj tiles interleaved ----
        oq = []
        evac_flip = [0]

        def emit_outproj(ypsp, youtp, n=1, force_act=False):
            for _ in range(n):
                if not oq:
                    return
                sb, dc = oq.pop(0)
                yps = ypsp.tile([128, SCW], F32, tag="yps")
                for p in range(2):
                    nc.tensor.matmul(
                        yps, ctxT[p][:, bass.ts(sb, 128)],
                        wo_t[:, p, bass.ts(dc, SCW)],
                        start=(p == 0), stop=(p == 1))
                yt = youtp.tile([128, SCW], BF16, tag="yt")
                if not force_act and evac_flip[0] % 3 != 2:
                    nc.vector.tensor_copy(out=yt, in_=yps)
                else:
                    nc.scalar.copy(yt, yps)
                evac_flip[0] += 1
                nc.sync.dma_start(
                    y_dram[bass.ts(sb, 128), bass.ts(dc, SCW)], yt)

        with tc.tile_pool(name="gps", bufs=2, space="PSUM") as gpsp, \
             tc.tile_pool(name="cps", bufs=1, space="PSUM") as cpsp, \
             tc.tile_pool(name="yps", bufs=2, space="PSUM") as ypsp, \
             tc.tile_pool(name="epool", bufs=3) as epool, \
             tc.tile_pool(name="yout", bufs=2) as youtp:
            for jc in range(SC):
                nkb = 4 * (jc + 1)
                for pack in range(2):
                    cps = cpsp.tile([65, 2, SCW], F32, tag="cps",
                                    name=f"cps{jc}_{pack}")
                    pend_ctx = None
                    for kb in range(nkb):
                        r = kb - 4 * jc
                        qlo = 128 * r if r >= 0 else 0
                        qsl = slice(qlo, SCW)
                        gsl = slice(jc * SCW + qlo, (jc + 1) * SCW)
                        gp = gpsp.tile([128, 2, SCW], F32, tag="gp",
                                       name=f"gp{jc}_{pack}_{kb}")
                        for h in range(2):
                            b = 64 * h
                            nc.tensor.matmul(
                                gp[:, h, qsl], krt[b:b + 64, bass.ts(kb, KBW)],
                                qrt[pack][b:b + 64, gsl],
                                start=True, stop=True, tile_position=(b, 0))
                        et = epool.tile([128, 2, SCW], BF16, tag="et")
                        nc.scalar.activation(et[:, :, qsl], gp[:, :, qsl],
                                             AF.Exp, scale=HD ** -0.5)
                        if r >= 0:
                            # zero the masked (key > query) triangle of the
                            # diagonal block post-exp, off the ACT chain
                            dsl = slice(qlo, qlo + KBW)
                            nc.vector.tensor_mul(et[:, :, dsl],
                                                 et[:, :, dsl], tri_t)
                        emit_outproj(ypsp, youtp)
                        if pend_ctx is not None:
                            pkb, pet, pqsl = pend_ctx
                            for h in range(2):
                                nc.tensor.matmul(
                                    cps[:, h, pqsl], vaug[:, pkb, :],
                                    pet[:, h, pqsl],
                                    start=(pkb == 0), stop=(pkb == nkb - 1))
                        pend_ctx = (kb, et, qsl)
                    pkb, pet, pqsl = pend_ctx
                    for h in range(2):
                        nc.tensor.matmul(
                            cps[:, h, pqsl], vaug[:, pkb, :], pet[:, h, pqsl],
                            start=(pkb == 0), stop=(pkb == nkb - 1))
                    # normalize: 1/denominator, broadcast on gpsimd.
                    # (custom ops need base partition 0 -> per-h staging)
                    rcps = []
                    for h in range(2):
                        dn = tmp.tile([1, SCW], F32, tag="adn", bufs=2,
                                      name=f"adn{jc}_{pack}_{h}")
                        nc.vector.tensor_copy(out=dn, in_=cps[64:65, h, :])
                        rcp = tmp.tile([1, SCW], F32, tag="arcp", bufs=2,
                                       name=f"arcp{jc}_{pack}_{h}")
                        nc.vector.reciprocal_approx_fast(rcp, dn)
                        rcps.append(rcp)
                    emit_outproj(ypsp, youtp, n=2)
                    for h in range(2):
                        bc_sb = tmp.tile([64, SCW], F32, tag="abc_sb", bufs=2,
                                         name=f"abcsb{jc}_{pack}_{h}")
                        nc.gpsimd.partition_broadcast(bc_sb, rcps[h])
                        nc.vector.tensor_mul(
                            ctxT[pack][64 * h:64 * h + 64, bass.ts(jc, SCW)],
                            cps[0:64, h, :], bc_sb)
                for sb in range(4 * jc, 4 * jc + 4):
                    for dc in range(D // SCW):
                        oq.append((sb, dc))
            emit_outproj(ypsp, youtp, n=len(oq), force_act=True)


_CACHE = {}


def _get_nc():
    if "nc" not in _CACHE:
        nc = bacc.Bacc("TRN2", target_bir_lowering=False, debug=False)
        _build(nc)
        nc.compile()
        _CACHE["nc"] = nc
    return _CACHE["nc"]


def _prep_core_inputs(c, x, cos, sin, wq, wk, wv, wo, qg, kg):
    """Host-side sharding + layout marshaling for core c."""
    f = np.float32
    hsl = slice(c * HPC * HD, (c + 1) * HPC * HD)
    gsl = slice(c * HD, (c + 1) * HD)

    wq_c = wq[hsl, :].reshape(HPC, HD, D)[:, PERM, :].reshape(HPC * HD, D)
    wk_c = wk[gsl, :][PERM, :]
    wv_c = wv[gsl, :]

    cos_p, sin_p = cos[:, PERM], sin[:, PERM]
    qg_p, kg_p = qg[PERM], kg[PERM]

    cgq = np.ascontiguousarray((cos_p * qg_p).T, dtype=f)
    sgq = np.ascontiguousarray((sin_p * (SIGN * qg_p[PARTNER])).T, dtype=f)
    cgk = np.ascontiguousarray((cos_p * kg_p).T, dtype=f)
    sgk = np.ascontiguousarray((sin_p * (SIGN * kg_p[PARTNER])).T, dtype=f)

    hsq = np.zeros((128, 2), f)
    hsq[0:64, 0] = 1.0 / HD
    hsq[64:128, 1] = 1.0 / HD
    hsk = np.full((64, 1), 1.0 / HD, f)
    bcq = np.zeros((2, 128), f)
    bcq[0, 0:64] = 1.0
    bcq[1, 64:128] = 1.0
    bck = np.ones((1, 64), f)

    # tri[p, h, q] = 1 where key p <= query q within a diagonal 128-block
    tri1 = (np.arange(128)[:, None] <= np.arange(KBW)[None, :]).astype(f)
    tri = np.concatenate([tri1, tri1], axis=1)  # [128, 2*KBW]

    bf = BF16NP
    return {
        "xT": np.ascontiguousarray(x[0].T).astype(bf),
        "wqT": np.ascontiguousarray(wq_c.T).astype(bf),
        "wkvT": np.ascontiguousarray(np.concatenate([wk_c, wv_c], 0).T).astype(bf),
        "woT": np.ascontiguousarray(wo[:, hsl].T).astype(bf),
        "cgq": cgq, "sgq": sgq, "cgk": cgk, "sgk": sgk,
        "hsq": hsq.astype(bf), "hsk": hsk.astype(bf),
        "bcq": bcq, "bck": bck,
        "tri": tri.astype(bf),
        "ident": np.eye(64, dtype=f).astype(bf),
    }


def kernel(x, mask, cos, sin, wq, wk, wv, wo, qg, kg, _trace=False):
    nc = _get_nc()
    in_maps = [
        _prep_core_inputs(c, np.asarray(x), np.asarray(cos), np.asarray(sin),
                          np.asarray(wq), np.asarray(wk), np.asarray(wv),
                          np.asarray(wo), np.asarray(qg), np.asarray(kg))
        for c in range(NCORES)
    ]
    res = run_bass_kernel_spmd(nc, in_maps, core_ids=list(range(NCORES)),
                               trace=_trace)
    _CACHE["last_results"] = res
    out = np.zeros((S, D), dtype=np.float64)
    for r in res.results:
        out += np.asarray(r["y"]).astype(np.float64)
    return out.astype(np.float32).reshape(B, S, D)
